# revision 10
# baseline (speedup 1.0000x reference)
"""Biased matrix-factorization batch scoring on 8 Trainium2 NeuronCores.

out[b] = 3.5 + user_biases[users[b]] + item_biases[items[b]]
         + dot(user_factors[users[b]], item_factors[items[b]])

Data-parallel over the batch (2048 elements per core), tables replicated in
every core's HBM.  Host packs both tables into one combined table of 66-wide
rows (user row = [uf | ub | 1], item row = [itf | 1 | ib + 3.5], items
offset by NUM_USERS) so the row-wise dot of the two gathered rows IS the
final answer.  Per core (raw Bass): one 8KB index DMA, 32 INDIRECT1D
gathers on the GpSimd SWDGE (one random 264B row per partition per op,
~1.4us each -- the hardware floor and the dominant cost), elementwise
multiply + grouped sum-reduction split in quarters that overlap the
gathers, one 8KB store.  Post-finalize surgery drops the main-block boot
barrier (every cross-engine dependency is semaphore-protected, so engines
free-run out of boot and the index DMA overlaps the other engines' startup)
and the unused-semaphore clear storm in the tail."""

import numpy as np

GLOBAL_AVERAGE = 3.5
NUM_USERS = 1_000_000
NUM_ITEMS = 100_000
F = 64
B = 16384
NCORES = 8
BC = B // NCORES
P = 128
G = BC // P  # 16
W = F + 2  # 66
NROW = 2 * G  # 32 gathers
NQ = 4  # compute quarters
R = G // NQ  # 4 elements per quarter per partition

_BUILD_CACHE = {}


def build_nc(num_users=NUM_USERS, num_items=NUM_ITEMS, w=W):
    key = (num_users, num_items, w)
    if key in _BUILD_CACHE:
        return _BUILD_CACHE[key]

    import concourse.bass as bass
    import concourse.mybir as mybir
    from concourse.bass import IndirectOffsetOnAxis

    ncat = num_users + num_items
    nc = bass.Bass()
    idx = nc.dram_tensor("idx", [BC * 2], mybir.dt.int32, kind="ExternalInput")
    cat = nc.dram_tensor("cat", [ncat, w], mybir.dt.float32, kind="ExternalInput")
    out = nc.dram_tensor("out", [BC], mybir.dt.float32, kind="ExternalOutput")

    # Slot j in [q*8, q*8+8): j%8 < 4 -> user row of element p*16+q*4+(j%4),
    # else item row of the same element.  Gather op k handles slot column k.
    with (
        nc.sbuf_tensor([P, NROW], mybir.dt.int32) as t_idx,
        nc.sbuf_tensor([P, NROW * w], mybir.dt.float32) as rows,
        nc.sbuf_tensor([P, G * w], mybir.dt.float32) as prod,
        nc.sbuf_tensor([P, G], mybir.dt.float32) as res,
        nc.semaphore() as s_idx,
        nc.semaphore() as s_q0,
        nc.semaphore() as s_q1,
        nc.semaphore() as s_q2,
        nc.semaphore() as s_q3,
        nc.semaphore() as s_v,
        nc.semaphore() as s_c,
        nc.semaphore() as s_o,
        nc.Block() as block,
    ):
        s_q = [s_q0, s_q1, s_q2, s_q3]

        @block.sync
        def _(sync):
            sync.dma_start(
                t_idx[:], idx[:].rearrange("(p j) -> p j", j=NROW)
            ).then_inc(s_idx, 16)
            sync.wait_ge(s_c, NQ)
            sync.dma_start(
                out[:].rearrange("(p g) -> p g", g=G), res[:]
            ).then_inc(s_o, 16)
            sync.wait_ge(s_o, 16)

        @block.gpsimd
        def _(g):
            g.wait_ge(s_idx, 16)
            for k in range(NROW):
                g.indirect_dma_start(
                    out=rows[:, k * w : (k + 1) * w],
                    out_offset=None,
                    in_=cat[:],
                    in_offset=IndirectOffsetOnAxis(ap=t_idx[:, k : k + 1], axis=0),
                ).then_inc(s_q[k // (2 * R)], 16)

        @block.vector
        def _(v):
            for q in range(NQ):
                lo = q * 2 * R * w
                v.wait_ge(s_q[q], 2 * R * 16)
                v.tensor_mul(
                    prod[:, q * R * w : (q + 1) * R * w],
                    rows[:, lo : lo + R * w],
                    rows[:, lo + R * w : lo + 2 * R * w],
                ).then_inc(s_v, 1)
                v.wait_ge(s_v, q + 1)
                v.reduce_sum(
                    res[:, q * R : (q + 1) * R],
                    prod[:, q * R * w : (q + 1) * R * w].rearrange(
                        "p (g w) -> p g w", w=w
                    ),
                    axis=mybir.AxisListType.X,
                ).then_inc(s_c, 1)

    nc.finalize()

    # Preamble surgery: the main-block boot barrier serializes all five
    # engines before any body instruction, but every cross-engine dependency
    # in this program is already semaphore-protected, so dropping it lets the
    # SP index DMA overlap the other engines' boot.  Also drop the const-tile
    # memsets (no activation/const usage).
    barrier_sem_ids = set()
    for bb in nc.m.functions[0].blocks:
        for ins in bb.instructions:
            si = ins.sync_info
            if si:
                for u in list(si.on_update or []) + list(si.on_wait or []):
                    if "barrier_" in (getattr(u, "ant_name", "") or ""):
                        barrier_sem_ids.add(u.id)
    for bb in nc.m.functions[0].blocks:
        if bb.name != "main":
            continue
        keep = []
        for ins in bb.instructions:
            tn = type(ins).__name__
            drop = tn == "InstMemset"
            si = ins.sync_info
            if not drop and si and tn in ("InstDrain", "InstEventSemaphore"):
                touches = any(
                    getattr(u, "id", None) in barrier_sem_ids
                    for u in list(si.on_update or []) + list(si.on_wait or [])
                )
                drop = touches
            if not drop:
                keep.append(ins)
        if len(keep) != len(bb.instructions):
            bb.instructions[:] = keep

    # Tail surgery: drop sem-clear EVENT_SEMAPHOREs for semaphores this
    # program never touches.
    used = set()
    for bb in nc.m.functions[0].blocks:
        for ins in bb.instructions:
            si = ins.sync_info
            if si:
                for u in list(si.on_update or []) + list(si.on_wait or []):
                    sid = getattr(u, "id", None)
                    if sid is not None:
                        used.add(sid)
    for bb in nc.m.functions[0].blocks:
        keep = []
        for ins in bb.instructions:
            drop = False
            if type(ins).__name__ == "InstEventSemaphore":
                si = ins.sync_info
                ups = list(si.on_update or []) if si else []
                ws = list(si.on_wait or []) if si else []
                if not ws and len(ups) == 1:
                    u = ups[0]
                    if (
                        getattr(u, "value", None) == 0
                        and getattr(u, "sem_op", None) in ("set", "assign", None)
                        and getattr(u, "id", -1) not in used
                    ):
                        drop = True
            if not drop:
                keep.append(ins)
        if len(keep) != len(bb.instructions):
            bb.instructions[:] = keep

    _BUILD_CACHE[key] = nc
    return nc


def make_cat(user_factors, item_factors, user_biases, item_biases):
    """Row u = [uf[u] | ub[u] | 1]; row num_users+i = [itf[i] | 1 | ib[i]+3.5]
    so the row-wise dot alone is the final answer."""
    nu, f = user_factors.shape
    ni = item_factors.shape[0]
    w = f + 2
    cat = np.empty((nu + ni, w), np.float32)
    cat[:nu, :f] = user_factors
    cat[:nu, f] = np.asarray(user_biases).reshape(nu)
    cat[:nu, f + 1] = 1.0
    cat[nu:, :f] = item_factors
    cat[nu:, f] = 1.0
    cat[nu:, f + 1] = np.asarray(item_biases).reshape(ni) + np.float32(GLOBAL_AVERAGE)
    return cat


def make_idx(users, items, num_users, ncores=NCORES):
    """flat[p*32 + q*8 + j] = user id of element p*16+q*4+j        (j<4)
                            = num_users + item id of elem p*16+q*4+j-4 (j>=4)"""
    u = np.asarray(users, dtype=np.int32).reshape(ncores, P, NQ, R)
    it = np.asarray(items, dtype=np.int32).reshape(ncores, P, NQ, R) + np.int32(
        num_users
    )
    inter = np.concatenate([u, it], axis=3)  # [ncores, P, NQ, 2R]
    return np.ascontiguousarray(inter.reshape(ncores, 2 * BC))


def kernel(users, items, user_factors, item_factors, user_biases, item_biases):
    from concourse.bass_utils import run_bass_kernel_spmd

    nc = build_nc()
    cat = make_cat(user_factors, item_factors, user_biases, item_biases)
    idx = make_idx(users, items, NUM_USERS)
    in_maps = [{"idx": idx[c], "cat": cat} for c in range(NCORES)]
    res = run_bass_kernel_spmd(nc, in_maps, core_ids=list(range(NCORES)))
    return np.concatenate([res.results[c]["out"] for c in range(NCORES)])
